# revision 33
# baseline (speedup 1.0000x reference)
"""Trainium2 Bass kernel for nn_Recommender_2 (moe_routing).

Pipeline per core (8 NeuronCores, one TRN2 chip):
  Phase 1 (data-parallel over batch, 128 rows/core):
    - indirect-DMA gather of ratings embeddings (table pre-cast to bf16)
    - PE-transposes (pipelined 4 ahead) to time-major x^T tiles
    - 2-layer LSTM scan (T=50) in transposed [gate, batch] layout:
      weights stationary (lhsT), state h^T flows as rhs -> no per-step
      transposes. Gate order host-permuted to (g,i,f,o); PSUM groups
      close per gate-pair. Sigmoids are computed AS TANH: the i/f/o
      weight rows are pre-halved on the host (sig(x) = 0.5*tanh(x/2) +
      0.5), so ACT runs just two 512-wide tanh instructions per cell
      (plus tanh(c)) -- 26us less ACT busy than tanh+2 sigmoids; the
      0.5t+0.5 affine folds into the DVE cell products via the
      affine_mul_reduce custom op. Gates AND the cell state c are bf16
      (costs ~1e-4 rel err). Layer-0 input
      matmuls are pipelined one step ahead into parity PSUM tiles (the
      x-gate filler writes the OTHER parity, so it never WAR-stalls the
      PE queue -- the PE streams ~56 back-to-back matmuls per step with
      a single idle gap, which keeps the HAM clock gate at 2.4 GHz);
      layer-1 runs one step delayed, its matmuls filling the PE while
      layer-0's activation chain runs. After the tanh conversion the
      scan is chain-bound (ACT ~67%, DVE ~64%): the serial
      mm->tanh->product->tanh(c)->h cycle is ~5.1us/step. Reordering
      the PE queue was measured SLOWER (idle fragmentation -> HAM
      throttle to 1.2 GHz). NOTE: run-to-run wall time varies +-40us
      with the board power governor (PE at 13/16 duty + uncore at full
      clock is the FAST state; PE 8/8 + uncore -25% is the slow one).
    - user MLP runs before the scan with its relu/copy on DVE (zero
      biases), so ACT sees ONLY Tanh for the whole kernel: one act-table
      load, no switches; the first x^T transposes are issued ahead of
      the u-MLP so the scan starts ~7us earlier. The z_u half is
      AllGathered DURING the scan; remote z_u halves are also gathered
      during the scan. Ratings MLP -> z_r^T, AllGathered at the end
      (it runs inside the expert PSUM region, behind two pre-issued
      expert z_u-chunk matmul groups that keep the PE streaming).
  Phase 2 (expert-parallel, 8 experts/core over full batch, bf16):
    - local-first: the core's own batch chunk is computed straight from
      the local zuT/zrT SBUF tiles while the z_r AllGather is in flight;
      the 7 remote chunks then fetch only their z_r halves via indirect
      DMA using a per-core chunk-order index tensor (the SPMD program
      has no core id). The host undoes the slot order on reassembly.
    - he = z @ W1_e accumulated in PSUM via 512-col chunks; the second
      GEMM (blocked by the relu) is one DVE scalar_tensor_tensor
      (he max 0)*w2 with accum_out per (expert, slot). All 8 experts'
      w2 rows (bf16) are PRELOADED during the scan: a per-item w2 DMA
      chain through a 2-buffer pool was the real phase-2 cadence
      limiter (measured STT interval 2.6us = DMA-bound, not PE).
  (fp8 DoubleRow for phase 2 was tried: 2x PE speed but e4m3 noise
  amplifies through the pos/neg cancellation in the w2 reduction to
  ~4e-2 rel err even with per-column/per-row scales -- over budget.)
Host reassembles [1024, 64] from per-core slot-ordered [1024, 8] outputs.
"""
import numpy as np
import ml_dtypes

import concourse.bacc as bacc
import concourse.bass as bass
import concourse.mybir as mybir
import concourse.tile as tile
from concourse.bass_utils import run_bass_kernel_spmd
from concourse.masks import make_identity

P = 128
NCORES = 8
B, T = 1024, 50
RV, RD, RH = 100000, 128, 256
R_OUT = 256
UV, UD, UDATA, U_OUT = 50000, 64, 32, 128
E, EIN, EH = 64, 384, 1536
EPC = E // NCORES
BL = B // NCORES
UIN = UD + UDATA
UH = 192
RHID = 512
NG = 8

F32 = mybir.dt.float32
BF16 = mybir.dt.bfloat16
FP8 = mybir.dt.float8e4
I32 = mybir.dt.int32
AF = mybir.ActivationFunctionType
ALU = mybir.AluOpType

# All experts reduce on DVE: relu(he)*w2 with accum_out, native hidden
# order (no sign-partition padding -> H2 == EH exactly). ACT stays free
# in phase 2; PE streaming is the binding engine.
N_W2 = EPC

_cache = {}


def _core_order(c):
    return [c] + [cc for cc in range(NCORES) if cc != c]


def _prep(inputs):
    f = lambda k: np.asarray(inputs[k], dtype=np.float32)
    bf = lambda a: np.ascontiguousarray(a, dtype=ml_dtypes.bfloat16)
    e4 = lambda a: np.ascontiguousarray(a, dtype=ml_dtypes.float8_e4m3)
    f32c = lambda a: np.ascontiguousarray(a, dtype=np.float32)

    # gate permutation i,f,g,o -> g,i,f,o  (bank0: g,i -> tanh(g) early,
    # then one sigmoid over i|f|o)
    perm = np.r_[512:768, 0:256, 256:512, 768:1024]

    shared = {}
    lstm_bias = False
    for l in range(2):
        bias = (f(f"lstm_b_ih_{l}") + f(f"lstm_b_hh_{l}"))[perm]
        lstm_bias = lstm_bias or bool(np.any(bias))
    for l in range(2):
        wih = f(f"lstm_W_ih_{l}")[perm]
        whh = f(f"lstm_W_hh_{l}")[perm]
        bias = (f(f"lstm_b_ih_{l}") + f(f"lstm_b_hh_{l}"))[perm]
        if not lstm_bias:
            # sigmoid-as-tanh: sig(x) = 0.5*tanh(x/2) + 0.5. Pre-halving
            # the i/f/o rows lets ACT run ALL gates as plain tanh -> two
            # 512-wide instructions per cell instead of tanh + 2 sigmoids;
            # the 0.5t+0.5 affine folds into the DVE cell products
            # (affine_mul_reduce custom op).
            wih[256:1024] *= 0.5
            whh[256:1024] *= 0.5
        shared[f"wih{l}"] = bf(wih.T)            # [in, 1024]
        shared[f"whh{l}"] = bf(whh.T)            # [256, 1024]
        shared[f"lb{l}"] = f32c(bias.reshape(NG, P).T)

    shared["rw1"] = bf(f("r_W1"))
    shared["rw2"] = bf(f("r_W2"))
    shared["uw1"] = bf(f("u_W1"))
    shared["uw2"] = bf(f("u_W2"))
    mlpb = np.zeros((P, 9), np.float32)
    mlpb[:, 0:4] = f("r_b1").reshape(4, P).T
    mlpb[:, 4:6] = f("r_b2").reshape(2, P).T
    ub1 = f("u_b1")
    mlpb[:, 6] = ub1[0:128]
    mlpb[0:64, 7] = ub1[128:192]
    mlpb[:, 8] = f("u_b2")
    mlp_bias = bool(np.any(mlpb))
    shared["mlpb"] = mlpb

    shared["remb"] = bf(f("ratings_emb"))        # bf16 table
    shared["uemb"] = bf(f("user_emb"))           # bf16 table

    w1 = f("exp_W1")                             # [64, 384, 1536]
    w2 = f("exp_W2").reshape(E, EH)              # [64, 1536]
    b1 = f("exp_b1")                             # [64, 1536]
    b2 = f("exp_b2").reshape(E)
    exp_b1_nz = bool(np.any(b1))

    ridx = np.asarray(inputs["ratings_tensor"]).astype(np.int32)
    uids = np.asarray(inputs["user_ids"]).astype(np.int32)
    udata = f("user_data")

    in_maps = []
    for c in range(NCORES):
        m = dict(shared)
        sl = slice(c * BL, (c + 1) * BL)
        m["ridx"] = np.ascontiguousarray(ridx[sl])
        m["uid"] = np.ascontiguousarray(uids[sl].reshape(BL, 1))
        m["udata"] = bf(udata[sl])
        es = slice(c * EPC, (c + 1) * EPC)
        m["expw"] = bf(w1[es])                   # [8, 384, 1536]
        m["w2r"] = bf(np.broadcast_to(
            w2[es][:, None, :], (N_W2, P, EH)))
        m["b1p"] = bf(b1[es])                    # [8, 1536]
        # slot-ordered b2 broadcast: col = slot*EPC + e -> b2[c*EPC+e]
        b2bc = np.zeros((P, E), np.float32)
        for slot in range(NCORES):
            for e in range(EPC):
                b2bc[:, slot * EPC + e] = b2[c * EPC + e]
        m["b2bc"] = b2bc
        # remote-chunk gather indices into the flattened Zall buffers
        rem = _core_order(c)[1:]
        zgi = np.empty((P, NCORES - 1), np.int32)
        for k, cc in enumerate(rem):
            zgi[:, k] = cc * P + np.arange(P, dtype=np.int32)
        m["zgi"] = zgi
        in_maps.append(m)

    bp = dict(lstm_bias=lstm_bias, mlp_bias=mlp_bias, exp_b1=exp_b1_nz,
              exp_b2=bool(np.any(b2)))
    return in_maps, bp


def _chunks(width):
    return [(s, min(s + 512, width)) for s in range(0, width, 512)]


def _build(bp, sim_single=False):

    nc = bacc.Bacc("TRN2", target_bir_lowering=False)
    d_ridx = nc.dram_tensor("ridx", [BL, T], I32, kind="ExternalInput")
    d_uid = nc.dram_tensor("uid", [BL, 1], I32, kind="ExternalInput")
    d_udata = nc.dram_tensor("udata", [BL, UDATA], BF16, kind="ExternalInput")
    d_remb = nc.dram_tensor("remb", [RV, RD], BF16, kind="ExternalInput")
    d_uemb = nc.dram_tensor("uemb", [UV, UD], BF16, kind="ExternalInput")
    d_wih = [nc.dram_tensor(f"wih{l}", [RD if l == 0 else RH, 4 * RH], BF16,
                            kind="ExternalInput") for l in range(2)]
    d_whh = [nc.dram_tensor(f"whh{l}", [RH, 4 * RH], BF16, kind="ExternalInput")
             for l in range(2)]
    d_lb = [nc.dram_tensor(f"lb{l}", [P, NG], F32, kind="ExternalInput")
            for l in range(2)]
    d_rw1 = nc.dram_tensor("rw1", [RH, RHID], BF16, kind="ExternalInput")
    d_rw2 = nc.dram_tensor("rw2", [RHID, R_OUT], BF16, kind="ExternalInput")
    d_uw1 = nc.dram_tensor("uw1", [UIN, UH], BF16, kind="ExternalInput")
    d_uw2 = nc.dram_tensor("uw2", [UH, U_OUT], BF16, kind="ExternalInput")
    d_mlpb = nc.dram_tensor("mlpb", [P, 9], F32, kind="ExternalInput")
    d_expw = nc.dram_tensor("expw", [EPC, EIN, EH], BF16, kind="ExternalInput")
    d_w2r = nc.dram_tensor("w2r", [N_W2, P, EH], BF16, kind="ExternalInput")
    d_b1p = nc.dram_tensor("b1p", [EPC, EH], BF16, kind="ExternalInput")
    d_b2bc = nc.dram_tensor("b2bc", [P, E], F32, kind="ExternalInput")
    d_zgi = nc.dram_tensor("zgi", [P, NCORES - 1], I32, kind="ExternalInput")
    d_out = nc.dram_tensor("out", [B, EPC], F32, kind="ExternalOutput")

    with tile.TileContext(nc) as tc:
        with (
            tc.tile_pool(name="sb", bufs=1) as sb,
            tc.tile_pool(name="dr", bufs=1, space="DRAM") as dr,
        ):
            # ---- latency-critical loads first (indices feed the gathers) ----
            ridx_t = sb.tile([BL, T], I32)
            nc.sync.dma_start(out=ridx_t[:], in_=d_ridx[:])
            uid_t = sb.tile([BL, 1], I32)
            nc.sync.dma_start(out=uid_t[:], in_=d_uid[:])
            zgi_t = sb.tile([P, NCORES - 1], I32)
            nc.sync.dma_start(out=zgi_t[:], in_=d_zgi[:])
            Uin = sb.tile([P, P], BF16)
            nc.gpsimd.memset(Uin[:, UIN:P], 0.0)
            nc.gpsimd.indirect_dma_start(
                out=Uin[:, 0:UD], out_offset=None, in_=d_uemb[:],
                in_offset=bass.IndirectOffsetOnAxis(ap=uid_t[:, 0:1], axis=0))
            nc.sync.dma_start(out=Uin[:, UD:UIN], in_=d_udata[:])
            X = sb.tile([P, T, RD], BF16)
            for t in range(T):
                nc.gpsimd.indirect_dma_start(
                    out=X[:, t, :], out_offset=None, in_=d_remb[:],
                    in_offset=bass.IndirectOffsetOnAxis(ap=ridx_t[:, t:t + 1], axis=0))

            # ---- small static weights ----
            wih_t = []
            whh_t = []
            for l in range(2):
                kin = RD if l == 0 else RH
                wt = []
                for kc in range(kin // P):
                    tl = sb.tile([P, 4 * RH], BF16, tag=f"wih{l}_{kc}")
                    nc.sync.dma_start(out=tl[:], in_=d_wih[l][kc * P:(kc + 1) * P, :])
                    wt.append(tl)
                wih_t.append(wt)
                ht = []
                for kc in range(2):
                    tl = sb.tile([P, 4 * RH], BF16, tag=f"whh{l}_{kc}")
                    nc.sync.dma_start(out=tl[:], in_=d_whh[l][kc * P:(kc + 1) * P, :])
                    ht.append(tl)
                whh_t.append(ht)
            lb_t = []
            for l in range(2):
                tl = sb.tile([P, NG], F32, tag=f"lb{l}")
                nc.sync.dma_start(out=tl[:], in_=d_lb[l][:])
                lb_t.append(tl)
            rw1_t = []
            for kc in range(2):
                tl = sb.tile([P, RHID], BF16, tag=f"rw1_{kc}")
                nc.sync.dma_start(out=tl[:], in_=d_rw1[kc * P:(kc + 1) * P, :])
                rw1_t.append(tl)
            rw2_t = []
            for kc in range(4):
                tl = sb.tile([P, R_OUT], BF16, tag=f"rw2_{kc}")
                nc.sync.dma_start(out=tl[:], in_=d_rw2[kc * P:(kc + 1) * P, :])
                rw2_t.append(tl)
            uw1_t = sb.tile([UIN, UH], BF16)
            nc.sync.dma_start(out=uw1_t[:], in_=d_uw1[:])
            uw2a = sb.tile([P, U_OUT], BF16)
            nc.sync.dma_start(out=uw2a[:], in_=d_uw2[0:P, :])
            uw2b = sb.tile([UH - P, U_OUT], BF16)
            nc.sync.dma_start(out=uw2b[:], in_=d_uw2[P:UH, :])
            mlpb_t = sb.tile([P, 9], F32)
            nc.sync.dma_start(out=mlpb_t[:], in_=d_mlpb[:])
            b2bc_t = sb.tile([P, E], F32)
            nc.sync.dma_start(out=b2bc_t[:], in_=d_b2bc[:])
            if bp["exp_b1"]:
                b1p_t = sb.tile([EPC, EH], BF16)
                nc.sync.dma_start(out=b1p_t[:], in_=d_b1p[:])
                ones1 = sb.tile([1, P], BF16)
                nc.gpsimd.memset(ones1[:], 1.0)
            identb = sb.tile([P, P], BF16)
            make_identity(nc, identb[:])
            # expert weights: big, needed only in phase 2 -> emitted last
            w1e_t = []
            for e in range(EPC):
                tl = sb.tile([P, EIN // P, EH], BF16, tag=f"w1e{e}")
                for i in range(EIN // P):
                    nc.sync.dma_start(out=tl[:, i, :],
                                      in_=d_expw[e, i * P:(i + 1) * P, :])
                w1e_t.append(tl)
            # all 8 experts' w2 rows preloaded during the scan: the per-item
            # w2 DMA chain (sw2 bufs=2) was the real phase-2 cadence limiter
            # (STT interval 2.6us = DMA-bound, not the PE)
            w2_t = []
            for e in range(EPC):
                tl = sb.tile([P, EH], BF16, tag=f"w2_{e}")
                nc.sync.dma_start(out=tl[:], in_=d_w2r[e])
                w2_t.append(tl)

            zuT = sb.tile([P, P], BF16)
            zrT = sb.tile([P, R_OUT], BF16)

            with (
                tc.tile_pool(name="ptm", bufs=1, space="PSUM") as ptm,
                tc.tile_pool(name="pXG", bufs=1, space="PSUM") as pXG,
                tc.tile_pool(name="pG1", bufs=1, space="PSUM") as pG1,
            ):
                # first x^T tiles before the u-MLP: the scan's first
                # step only waits these + the u-MLP's 5 small matmuls
                XT = sb.tile([P, T, RD], BF16)

                def transpose_x(t):
                    tr = ptm.tile([P, P], BF16, name="tr", tag="tm")
                    nc.tensor.transpose(out=tr[:], in_=X[:, t, :],
                                        identity=identb[:])
                    nc.vector.tensor_copy(out=XT[:, t, :], in_=tr[:])

                for t in range(4):
                    transpose_x(t)

                # ---- user MLP (independent of LSTM) ----
                tru = ptm.tile([P, P], BF16, tag="tm")
                nc.tensor.transpose(out=tru[:], in_=Uin[:], identity=identb[:])
                UinT = sb.tile([P, P], BF16)
                nc.vector.tensor_copy(out=UinT[:], in_=tru[:])
                u1ps = ptm.tile([P, 2 * P], F32, tag="tm")
                nc.tensor.matmul(out=u1ps[:, 0:P], lhsT=uw1_t[:, 0:P],
                                 rhs=UinT[0:UIN, :], start=True, stop=True)
                nc.tensor.matmul(out=u1ps[0:UH - P, P:2 * P], lhsT=uw1_t[:, P:UH],
                                 rhs=UinT[0:UIN, :], start=True, stop=True)
                U1T = sb.tile([P, 2 * P], BF16)
                if bp["mlp_bias"]:
                    nc.scalar.activation(U1T[:, 0:P], u1ps[:, 0:P], AF.Relu,
                                         bias=mlpb_t[:, 6:7])
                    nc.scalar.activation(U1T[0:UH - P, P:2 * P],
                                         u1ps[0:UH - P, P:2 * P],
                                         AF.Relu, bias=mlpb_t[0:UH - P, 7:8])
                else:
                    # relu on DVE: keeps ACT entirely out of the pre-scan
                    # window -> the Tanh act table loads once and never
                    # switches (each ACT_TABLE_LOAD is 1.28us)
                    nc.vector.tensor_scalar(out=U1T[:, 0:P],
                                            in0=u1ps[:, 0:P],
                                            scalar1=0.0, scalar2=None,
                                            op0=ALU.max)
                    nc.vector.tensor_scalar(out=U1T[0:UH - P, P:2 * P],
                                            in0=u1ps[0:UH - P, P:2 * P],
                                            scalar1=0.0, scalar2=None,
                                            op0=ALU.max)
                u2ps = ptm.tile([P, P], F32, tag="tm")
                nc.tensor.matmul(out=u2ps[:], lhsT=uw2a[:], rhs=U1T[:, 0:P],
                                 start=True, stop=False)
                nc.tensor.matmul(out=u2ps[:], lhsT=uw2b[:], rhs=U1T[0:UH - P, P:2 * P],
                                 start=False, stop=True)
                if bp["mlp_bias"]:
                    nc.scalar.activation(zuT[:], u2ps[:], AF.Identity,
                                         bias=mlpb_t[:, 8:9])
                else:
                    nc.vector.tensor_copy(out=zuT[:], in_=u2ps[:])

                zu_dr = dr.tile([P, P], BF16)
                nc.sync.dma_start(out=zu_dr[:], in_=zuT[:])
                Zall_u = dr.tile([NCORES * P, P], BF16,
                                 addr_space="Local" if sim_single else "Shared")
                if sim_single:
                    for cc in range(NCORES):
                        nc.sync.dma_start(out=Zall_u[cc * P:(cc + 1) * P, :],
                                          in_=zu_dr[:])
                else:
                    nc.gpsimd.collective_compute(
                        "AllGather", ALU.bypass, ins=[zu_dr.opt()],
                        outs=[Zall_u.opt()],
                        replica_groups=[list(range(NCORES))])
                # remote z_u halves: gather DURING the scan (u-collective
                # lands ~100us before z_r exists; DMA is idle then)
                ztR = []
                for k in range(NCORES - 1):
                    tl = sb.tile([P, EIN], BF16, tag=f"ztr{k}")
                    nc.gpsimd.indirect_dma_start(
                        out=tl[:, 0:P], out_offset=None, in_=Zall_u[:],
                        in_offset=bass.IndirectOffsetOnAxis(
                            ap=zgi_t[:, k:k + 1], axis=0))
                    ztR.append(tl)

                # ---- LSTM scan ----
                XG = [pXG.tile([P, 4 * RH], F32, name="XG0"),
                      pXG.tile([P, 4 * RH], F32, name="XG1")]
                G1 = pG1.tile([P, 4 * RH], F32, name="G1")
                # S layout: [tanh_g | sig_i | sig_f | sig_o], 256 each, bf16
                # (pure-bf16 SBUF operands unlock the DVE 4x mode); c stays
                # fp32 (it accumulates across the 50 steps)
                # S and TC double-buffered by step parity: step t's gate
                # writes otherwise WAR-wait on step t-1's h-mult reads
                S = [[sb.tile([P, 1024], BF16, name=f"S{l}_{j}")
                      for j in range(2)] for l in range(2)]
                # c in bf16: costs ~1e-4 rel err but every cell DVE op
                # becomes all-bf16 SBUF -> DVE 4x mode incl t2 and cw
                C = [[sb.tile([P, RH], BF16, name=f"C{l}_{j}")
                      for j in range(2)] for l in range(2)]
                TT1 = [sb.tile([P, RH], BF16, name=f"TT1{l}") for l in range(2)]
                TT2 = [sb.tile([P, RH], BF16, name=f"TT2{l}") for l in range(2)]
                TC = [[sb.tile([P, RH], BF16, name=f"TC{l}_{j}")
                       for j in range(2)] for l in range(2)]
                # h0 double-buffered: layer-1 (delayed one step) still needs
                # h0(t-1) after cell(0,t) has produced h0(t)
                hT0 = [sb.tile([P, RH], BF16, name=f"hT0_{j}") for j in range(2)]
                # hT1 double-buffered too: the h1 DVE write would otherwise
                # WAR-wait on all 16 whh1 matmul reads and can clog the DVE
                # queue head past the engines' 4-deep OOO window
                hT1 = [sb.tile([P, RH], BF16, name=f"hT1_{j}") for j in range(2)]

                def cell_gates(l, t, g):
                    # tanh(g) after the first gate-pair group closes;
                    # sigmoid split (i,f | o) so t2 = f*c starts before
                    # sigmoid(o) occupies the ACT engine
                    s = S[l][t % 2]
                    if bp["lstm_bias"]:
                        for jg in (0, 1):
                            nc.scalar.activation(s[:, jg * P:(jg + 1) * P],
                                                 g[:, jg * P:(jg + 1) * P],
                                                 AF.Tanh,
                                                 bias=lb_t[l][:, jg:jg + 1])
                        for jg in range(2, 8):
                            nc.scalar.activation(s[:, jg * P:(jg + 1) * P],
                                                 g[:, jg * P:(jg + 1) * P],
                                                 AF.Sigmoid,
                                                 bias=lb_t[l][:, jg:jg + 1])
                    else:
                        # all-tanh gates (i/f/o pre-halved in the weights):
                        # each 512-wide instr closes after 8 recurrent mms
                        nc.scalar.activation(s[:, 0:512], g[:, 0:512], AF.Tanh)
                        nc.scalar.activation(s[:, 512:1024], g[:, 512:1024],
                                             AF.Tanh)

                dacc = [[sb.tile([P, 1], F32, name=f"da{q}_{k}")
                         for k in range(3)] for q in range(4)]

                def cell_tail(l, t, g, h):
                    # c(t) lands in the parity buffer: the ADD write would
                    # otherwise WAR-wait on tanh-c(t-1)'s ACT read
                    s, tc_ = S[l][t % 2], TC[l][t % 2]
                    cw, cr = C[l][t % 2], C[l][(t - 1) % 2]
                    da = dacc[2 * l + (t % 2)]
                    if bp["lstm_bias"]:
                        if t == 0:
                            nc.vector.tensor_tensor(out=cw[:, :],
                                                    in0=s[:, 256:512],
                                                    in1=s[:, 0:256],
                                                    op=ALU.mult)
                        else:
                            nc.vector.tensor_tensor(out=TT2[l][:, :],
                                                    in0=s[:, 512:768],
                                                    in1=cr[:, :], op=ALU.mult)
                            nc.vector.tensor_tensor(out=TT1[l][:, :],
                                                    in0=s[:, 256:512],
                                                    in1=s[:, 0:256],
                                                    op=ALU.mult)
                            nc.vector.tensor_tensor(out=cw[:, :],
                                                    in0=TT1[l][:, :],
                                                    in1=TT2[l][:, :],
                                                    op=ALU.add)
                        nc.scalar.activation(tc_[:, :], cw[:, :], AF.Tanh)
                        nc.vector.tensor_tensor(out=h[:, :], in0=s[:, 768:1024],
                                                in1=tc_[:, :], op=ALU.mult)
                        return
                    # S holds tanh(x/2) for i/f/o: sig = 0.5*t + 0.5 is
                    # folded into the products as (in0*0.5+0.5)*in1
                    if t == 0:
                        nc.vector.affine_mul_reduce(
                            out=cw[:, :], accum_out=da[0][:],
                            in0=s[:, 256:512], in1=s[:, 0:256],
                            scale=0.5, bias=0.5)
                    else:
                        # t1 first: it needs only tanh_A (g,i); t2 waits
                        # tanh_B (f) which lands one ACT instr later
                        nc.vector.affine_mul_reduce(
                            out=TT1[l][:, :], accum_out=da[1][:],
                            in0=s[:, 256:512], in1=s[:, 0:256],
                            scale=0.5, bias=0.5)
                        nc.vector.affine_mul_reduce(
                            out=TT2[l][:, :], accum_out=da[0][:],
                            in0=s[:, 512:768], in1=cr[:, :],
                            scale=0.5, bias=0.5)
                        nc.vector.tensor_tensor(out=cw[:, :], in0=TT1[l][:, :],
                                                in1=TT2[l][:, :], op=ALU.add)
                    nc.scalar.activation(tc_[:, :], cw[:, :], AF.Tanh)
                    nc.vector.affine_mul_reduce(
                        out=h[:, :], accum_out=da[2][:],
                        in0=s[:, 768:1024], in1=tc_[:, :],
                        scale=0.5, bias=0.5)

                def cell(l, t, g, h):
                    cell_gates(l, t, g)
                    cell_tail(l, t, g, h)

                # PSUM group discipline: start=True clears has_written for the
                # WHOLE bank, so open each bank's group only on its first
                # slice. Groups CLOSE per gate-pair (jg 1,3,5,7) so tanh(g)
                # fires after only 4 recurrent matmuls.
                bank_first = lambda jg: jg % 4 == 0
                pair_last = lambda jg: jg % 2 == 1

                def l1_mms(u):
                    """layer-1 MMs for step u (issued one step late: at
                    issue time h0(u) is long ready -> no PE wait)."""
                    h0u = hT0[u % 2]
                    for jg in range(NG):
                        for kc in range(2):
                            nc.tensor.matmul(
                                out=G1[:, jg * P:(jg + 1) * P],
                                lhsT=wih_t[1][kc][:, jg * P:(jg + 1) * P],
                                rhs=h0u[:, kc * P:(kc + 1) * P],
                                start=(kc == 0 and bank_first(jg)),
                                stop=(u == 0 and kc == 1 and pair_last(jg)),
                                skip_group_check=True)
                    if u > 0:
                        for jg in range(NG):
                            for kc in range(2):
                                nc.tensor.matmul(
                                    out=G1[:, jg * P:(jg + 1) * P],
                                    lhsT=whh_t[1][kc][:, jg * P:(jg + 1) * P],
                                    rhs=hT1[(u - 1) % 2][:, kc * P:(kc + 1) * P],
                                    start=False,
                                    stop=(kc == 1 and pair_last(jg)),
                                    skip_group_check=True)

                # prologue: xg0 for t=0
                for jg in range(NG):
                    nc.tensor.matmul(out=XG[0][:, jg * P:(jg + 1) * P],
                                     lhsT=wih_t[0][0][:, jg * P:(jg + 1) * P],
                                     rhs=XT[:, 0, :], start=bank_first(jg),
                                     stop=pair_last(jg),
                                     skip_group_check=True)

                for t in range(T):
                    Gx = XG[t % 2]
                    # L0 recurrent (the critical chain); groups close per
                    # gate-pair so tanh(g) fires after 4 matmuls
                    if t > 0:
                        for jg in range(NG):
                            for kc in range(2):
                                nc.tensor.matmul(
                                    out=Gx[:, jg * P:(jg + 1) * P],
                                    lhsT=whh_t[0][kc][:, jg * P:(jg + 1) * P],
                                    rhs=hT0[(t - 1) % 2][:, kc * P:(kc + 1) * P],
                                    start=False,
                                    stop=(kc == 1 and pair_last(jg)),
                                    skip_group_check=True)
                    cell(0, t, Gx, hT0[t % 2])
                    # layer 1 for the previous step: all operands ready
                    if t > 0:
                        l1_mms(t - 1)
                        cell(1, t - 1, G1, hT1[(t - 1) % 2])
                    # filler: xg0 for step t+1 (keeps PE warm, off-chain)
                    if t + 1 < T:
                        Gn = XG[(t + 1) % 2]
                        for jg in range(NG):
                            nc.tensor.matmul(
                                out=Gn[:, jg * P:(jg + 1) * P],
                                lhsT=wih_t[0][0][:, jg * P:(jg + 1) * P],
                                rhs=XT[:, t + 1, :], start=bank_first(jg),
                                stop=False, skip_group_check=True)
                    if t + 4 < T:
                        transpose_x(t + 4)
                l1_mms(T - 1)
                cell(1, T - 1, G1, hT1[(T - 1) % 2])

            # ---- ratings MLP + experts (single PSUM region) ----
            # The r-MLP runs here, AFTER the scan pools close, so the first
            # experts' z_u-chunk matmuls (ready: zuT + w1e only) can be
            # issued ahead of it -- the PE streams through the r-MLP's
            # serial DVE/copy chain and stays warm into the local pass
            # (previously it cooled to 1.2 GHz for ~25us at this boundary).
            with (
                tc.tile_pool(name="phe", bufs=2, space="PSUM") as phe,
                tc.tile_pool(name="prm", bufs=1, space="PSUM") as prm,
            ):
                n_rem = NCORES - 1
                loc_z = [zuT[:, 0:P], zrT[:, 0:P], zrT[:, P:2 * P]]
                scrd = sb.tile([P, EH], BF16)
                souts = sb.tile([P, E], F32)
                outs = sb.tile([P, E], F32)
                chunks = _chunks(EH)

                def expert_he_i0(e):
                    # z_u contribution only (chunk i=0): opens the groups
                    he = phe.tile([P, EH], F32, name="he", tag="he")
                    for (n0, n1) in chunks:
                        nc.tensor.matmul(out=he[:, n0:n1], lhsT=zuT[:, 0:P],
                                         rhs=w1e_t[e][:, 0, n0:n1],
                                         start=True, stop=False)
                    return he

                def expert_he_rest(he, e, zsl):
                    for i in range(1, EIN // P):
                        last = (i == EIN // P - 1) and not bp["exp_b1"]
                        for (n0, n1) in chunks:
                            nc.tensor.matmul(
                                out=he[:, n0:n1], lhsT=zsl[i],
                                rhs=w1e_t[e][:, i, n0:n1],
                                start=False, stop=last)
                    if bp["exp_b1"]:
                        for (n0, n1) in chunks:
                            nc.tensor.matmul(
                                out=he[:, n0:n1], lhsT=ones1[:],
                                rhs=b1p_t[e:e + 1, n0:n1],
                                start=False, stop=True)

                def expert_he(e, zsl):
                    he = phe.tile([P, EH], F32, name="he", tag="he")
                    for (n0, n1) in chunks:
                        nc.tensor.matmul(out=he[:, n0:n1], lhsT=zsl[0],
                                         rhs=w1e_t[e][:, 0, n0:n1],
                                         start=True, stop=False)
                    expert_he_rest(he, e, zsl)
                    return he

                def slot_out(slot):
                    # per-slot bias add + output DMA as soon as the slot's
                    # last expert reduces (shortens the kernel tail)
                    c0, c1 = slot * EPC, (slot + 1) * EPC
                    if bp["exp_b2"]:
                        nc.vector.tensor_tensor(out=outs[:, c0:c1],
                                                in0=souts[:, c0:c1],
                                                in1=b2bc_t[:, c0:c1],
                                                op=ALU.add)
                        fin = outs
                    else:
                        fin = souts
                    nc.sync.dma_start(out=d_out[slot * P:(slot + 1) * P, :],
                                      in_=fin[:, c0:c1])

                # warm-fill before the r-MLP
                pend = [expert_he_i0(0), expert_he_i0(1)]

                r1ps = prm.tile([P, RHID], F32, tag="rm")
                nmm = 0
                for mc in range(4):
                    for kc in range(2):
                        nmm += 1
                        nc.tensor.matmul(
                            out=r1ps[:, mc * P:(mc + 1) * P],
                            lhsT=rw1_t[kc][:, mc * P:(mc + 1) * P],
                            rhs=hT1[(T - 1) % 2][:, kc * P:(kc + 1) * P],
                            start=(nmm == 1), stop=(nmm == 8))
                R1T = sb.tile([P, RHID], BF16)
                if bp["mlp_bias"]:
                    for mc in range(4):
                        nc.scalar.activation(R1T[:, mc * P:(mc + 1) * P],
                                             r1ps[:, mc * P:(mc + 1) * P], AF.Relu,
                                             bias=mlpb_t[:, mc:mc + 1])
                else:
                    # relu on DVE: avoids an ACT table switch (Relu is in a
                    # different act table than the scan's Tanh) on the
                    # z-critical tail
                    nc.vector.tensor_scalar(out=R1T[:], in0=r1ps[:],
                                            scalar1=0.0, scalar2=None,
                                            op0=ALU.max)
                r2ps = prm.tile([P, R_OUT], F32, tag="rm")
                nmm = 0
                for mc in range(2):
                    for kc in range(4):
                        nmm += 1
                        nc.tensor.matmul(
                            out=r2ps[:, mc * P:(mc + 1) * P],
                            lhsT=rw2_t[kc][:, mc * P:(mc + 1) * P],
                            rhs=R1T[:, kc * P:(kc + 1) * P],
                            start=(nmm == 1), stop=(nmm == 8))
                if bp["mlp_bias"]:
                    for mc in range(2):
                        nc.scalar.activation(zrT[:, mc * P:(mc + 1) * P],
                                             r2ps[:, mc * P:(mc + 1) * P],
                                             AF.Identity,
                                             bias=mlpb_t[:, 4 + mc:5 + mc])
                else:
                    nc.vector.tensor_copy(out=zrT[:], in_=r2ps[:])

                # ---- allgather z_r (the u half went out during p1) ----
                zr_dr = dr.tile([P, R_OUT], BF16)
                nc.sync.dma_start(out=zr_dr[:], in_=zrT[:])
                Zall_r = dr.tile([NCORES * P, R_OUT], BF16,
                                 addr_space="Local" if sim_single else "Shared")
                if sim_single:
                    for cc in range(NCORES):
                        nc.sync.dma_start(out=Zall_r[cc * P:(cc + 1) * P, :],
                                          in_=zr_dr[:])
                else:
                    nc.gpsimd.collective_compute(
                        "AllGather", ALU.bypass, ins=[zr_dr.opt()],
                        outs=[Zall_r.opt()],
                        replica_groups=[list(range(NCORES))])
                for k in range(n_rem):
                    nc.gpsimd.indirect_dma_start(
                        out=ztR[k][:, P:EIN], out_offset=None, in_=Zall_r[:],
                        in_offset=bass.IndirectOffsetOnAxis(
                            ap=zgi_t[:, k:k + 1], axis=0))

                # pass 1: local chunk (covers the AllGather); i=0 chunks
                # of item e+2 are pre-issued so the PE never waits the STT
                for e in range(EPC):
                    he = pend.pop(0)
                    expert_he_rest(he, e, loc_z)
                    nc.vector.scalar_tensor_tensor(
                        out=scrd[:], in0=he[:], scalar=0.0,
                        in1=w2_t[e][:], op0=ALU.max, op1=ALU.mult,
                        accum_out=souts[:, e:e + 1])
                    if e + 2 < EPC:
                        pend.append(expert_he_i0(e + 2))
                slot_out(0)
                # pass 2: remote chunks, expert-major
                for e in range(EPC):
                    for k in range(n_rem):
                        zsl = [ztR[k][:, i * P:(i + 1) * P]
                               for i in range(EIN // P)]
                        he = expert_he(e, zsl)
                        col = (k + 1) * EPC + e
                        nc.vector.scalar_tensor_tensor(
                            out=scrd[:], in0=he[:], scalar=0.0,
                            in1=w2_t[e][:], op0=ALU.max, op1=ALU.mult,
                            accum_out=souts[:, col:col + 1])
                for k in range(n_rem):
                    slot_out(k + 1)
    nc.finalize()
    return nc


def _get_nc(bp, sim_single=False):
    key = (bp["lstm_bias"], bp["mlp_bias"], bp["exp_b1"], bp["exp_b2"],
           sim_single)
    if key not in _cache:
        _cache[key] = _build(bp, sim_single=sim_single)
    return _cache[key]


def run(inputs, trace=False):
    in_maps, bp = _prep(inputs)
    nc = _get_nc(bp)
    res = run_bass_kernel_spmd(nc, in_maps, core_ids=list(range(NCORES)),
                               trace=trace)
    out = np.empty((B, E), np.float32)
    for c in range(NCORES):
        o = np.asarray(res.results[c]["out"]).astype(np.float32)
        for slot, cc in enumerate(_core_order(c)):
            out[cc * BL:(cc + 1) * BL, c * EPC:(c + 1) * EPC] = \
                o[slot * BL:(slot + 1) * BL]
    return out, res


def kernel(**inputs) -> np.ndarray:
    out, _ = run(inputs, trace=False)
    return out

